# revision 7
# baseline (speedup 1.0000x reference)
"""Trainium2 Bass kernel for nn_BatchMoEDecoder.

Sharding: expert-parallel phase (2 experts/core over the full batch,
features-on-partitions matmul chain) -> AllToAll of expert outputs
(each core receives all 16 experts for its 128-row batch slice) ->
data-parallel tail (gate MLP with decomposed concat, top-2 routing,
one-hot combine matmul, numerical/categorical decode).

Self-contained: hardcodes all shapes from the problem spec.
"""
import sys

for _p in ("/opt/trn_rl_repo", "/root/.axon_site/_ro/trn_rl_repo"):
    if _p not in sys.path:
        sys.path.append(_p)

import numpy as np

import concourse.bass as bass
import concourse.mybir as mybir
import concourse.tile as tile
from concourse import bacc
from concourse.bass_utils import run_bass_kernel_spmd

B, S, C, E, H, D, V, K = 1024, 64, 512, 16, 1024, 256, 100, 2
NUM = 32
NCORES = 8
BL = B // NCORES      # 128 local batch rows
EL = E // NCORES      # 2 local experts
T = BL * S            # 8192 local tokens
F32 = mybir.dt.float32
I32 = mybir.dt.int32

AluOp = mybir.AluOpType
Act = mybir.ActivationFunctionType


def A(t, offset, pairs):
    return bass.AP(t, offset, pairs)


F32R = mybir.dt.float32r


def _build_nc():
    nc = bacc.Bacc("TRN2", target_bir_lowering=False, debug=False,
                   num_devices=NCORES)

    def mmr(out, lhsT, rhs, start, stop):
        # fp32r: full-rate PE pass for fp32 data (moving dim >= 256)
        nc.tensor.matmul(out, lhsT, rhs, start=start, stop=stop)

    # ---- DRAM I/O ----
    d_codecT = nc.dram_tensor("codecT", [C, B], F32, kind="ExternalInput")
    d_cmyT = nc.dram_tensor("codec_myT", [C, BL], F32, kind="ExternalInput")
    d_maskf = nc.dram_tensor("maskf", [BL, S], F32, kind="ExternalInput")
    d_pptT = nc.dram_tensor("pptT", [256, S], F32, kind="ExternalInput")
    d_gW1c = nc.dram_tensor("gW1c", [C, 256], F32, kind="ExternalInput")
    d_gW1m = nc.dram_tensor("gW1m", [1, 256], F32, kind="ExternalInput")
    d_gW2 = nc.dram_tensor("gW2", [256, E], F32, kind="ExternalInput")
    d_gb2 = nc.dram_tensor("gb2", [1, E], F32, kind="ExternalInput")
    d_ew0 = nc.dram_tensor("ew0", [EL, C, H], F32, kind="ExternalInput")
    d_eb0 = nc.dram_tensor("eb0", [EL, H], F32, kind="ExternalInput")
    d_ew1 = nc.dram_tensor("ew1", [EL, H, H], F32, kind="ExternalInput")
    d_eb1 = nc.dram_tensor("eb1", [EL, H], F32, kind="ExternalInput")
    d_ew2 = nc.dram_tensor("ew2", [EL, H, D], F32, kind="ExternalInput")
    d_eb2 = nc.dram_tensor("eb2", [EL, D], F32, kind="ExternalInput")
    d_nWT = nc.dram_tensor("nWT", [D, NUM], F32, kind="ExternalInput")
    d_nb = nc.dram_tensor("nb", [1, NUM], F32, kind="ExternalInput")
    d_cW = nc.dram_tensor("cW", [NUM, D, V], F32, kind="ExternalInput")
    d_cbT = nc.dram_tensor("cbT", [V, NUM], F32, kind="ExternalInput")

    d_numT = nc.dram_tensor("numT_out", [NUM, BL], F32, kind="ExternalOutput")
    d_catT = nc.dram_tensor("catT_out", [V, NUM, BL], F32, kind="ExternalOutput")
    d_g1o = nc.dram_tensor("g1_out", [BL, S], F32, kind="ExternalOutput")
    d_g2o = nc.dram_tensor("g2_out", [BL, S], F32, kind="ExternalOutput")
    d_i1o = nc.dram_tensor("i1_out", [BL, S], I32, kind="ExternalOutput")
    d_i2o = nc.dram_tensor("i2_out", [BL, S], I32, kind="ExternalOutput")

    # internal DRAM
    d_a2a_in = nc.dram_tensor("a2a_in", [NCORES, EL, 2, 128, 128], F32)
    d_a2a_out = nc.dram_tensor("a2a_out", [NCORES, EL, 2, 128, 128], F32)
    d_i1s = nc.dram_tensor("i1s", [BL, S], F32)
    d_i2s = nc.dram_tensor("i2s", [BL, S], F32)
    d_g1s = nc.dram_tensor("g1s", [BL, S], F32)
    d_g2s = nc.dram_tensor("g2s", [BL, S], F32)

    groups = [list(range(NCORES))]

    with tile.TileContext(nc) as tc:
        # =============== Scope 1: expert MLPs (full batch) ===============
        with tc.tile_pool(name="ecst", bufs=1) as ecst, \
             tc.tile_pool(name="ew", bufs=1) as ewp, \
             tc.tile_pool(name="eact", bufs=1) as eap, \
             tc.tile_pool(name="eout", bufs=2) as eop, \
             tc.tile_pool(name="epsum", bufs=2, space="PSUM") as eps:

            cT = ecst.tile([128, 4, B], F32R)   # codecT [c-chunk, kc, b]
            nc.sync.dma_start(out=cT, in_=A(d_codecT, 0, [[B, 128], [128 * B, 4], [1, B]]).bitcast(F32R))
            ebias0 = ecst.tile([128, EL, 8], F32)
            nc.sync.dma_start(out=ebias0, in_=A(d_eb0, 0, [[1, 128], [H, EL], [128, 8]]))
            ebias1 = ecst.tile([128, EL, 8], F32)
            nc.sync.dma_start(out=ebias1, in_=A(d_eb1, 0, [[1, 128], [H, EL], [128, 8]]))
            ebias2 = ecst.tile([128, EL, 2], F32)
            nc.sync.dma_start(out=ebias2, in_=A(d_eb2, 0, [[1, 128], [D, EL], [128, 2]]))

            for el in range(EL):
                w0 = ewp.tile([128, 4, H], F32R, tag="w0")
                nc.sync.dma_start(out=w0, in_=A(d_ew0, el * C * H,
                                                [[H, 128], [128 * H, 4], [1, H]]).bitcast(F32R))
                w1 = ewp.tile([128, 8, H], F32R, tag="w1")
                nc.sync.dma_start(out=w1, in_=A(d_ew1, el * H * H,
                                                [[H, 128], [128 * H, 8], [1, H]]).bitcast(F32R))
                w2 = ewp.tile([128, 8, D], F32R, tag="w2")
                nc.sync.dma_start(out=w2, in_=A(d_ew2, el * H * D,
                                                [[D, 128], [128 * D, 8], [1, D]]).bitcast(F32R))

                h0T = eap.tile([128, 8, B], F32R, tag="h0T")
                for hp in range(8):
                    ps = eps.tile([128, B], F32, tag="ps")
                    for bh in range(2):
                        for kc in range(4):
                            mmr(ps[:, bh * 512:(bh + 1) * 512],
                                w0[:, kc, hp * 128:(hp + 1) * 128],
                                cT[:, kc, bh * 512:(bh + 1) * 512],
                                start=(kc == 0), stop=(kc == 3))
                    # bias + exact leaky relu: max(x+b, 0.01*(x+b))
                    nc.vector.tensor_scalar_add(h0T[:, hp, :], ps,
                                                ebias0[:, el, hp:hp + 1])
                    nc.vector.scalar_tensor_tensor(
                        h0T[:, hp, :], h0T[:, hp, :], 0.01, h0T[:, hp, :],
                        op0=AluOp.mult, op1=AluOp.max)

                h1T = eap.tile([128, 8, B], F32R, tag="h1T")
                for gp in range(8):
                    ps = eps.tile([128, B], F32, tag="ps")
                    for bh in range(2):
                        for kc in range(8):
                            mmr(ps[:, bh * 512:(bh + 1) * 512],
                                w1[:, kc, gp * 128:(gp + 1) * 128],
                                h0T[:, kc, bh * 512:(bh + 1) * 512],
                                start=(kc == 0), stop=(kc == 7))
                    nc.vector.tensor_scalar_add(h1T[:, gp, :], ps,
                                                ebias1[:, el, gp:gp + 1])
                    nc.vector.scalar_tensor_tensor(
                        h1T[:, gp, :], h1T[:, gp, :], 0.01, h1T[:, gp, :],
                        op0=AluOp.mult, op1=AluOp.max)

                for dp in range(2):
                    ps = eps.tile([128, B], F32, tag="ps")
                    for bh in range(2):
                        for kc in range(8):
                            mmr(ps[:, bh * 512:(bh + 1) * 512],
                                w2[:, kc, dp * 128:(dp + 1) * 128],
                                h1T[:, kc, bh * 512:(bh + 1) * 512],
                                start=(kc == 0), stop=(kc == 7))
                    eo = eop.tile([128, B], F32, tag="eo")
                    nc.vector.tensor_scalar_add(eo, ps, ebias2[:, el, dp:dp + 1])
                    # scatter b-slices to a2a_in[j, el, dp, :, :]
                    nc.sync.dma_start(
                        out=A(d_a2a_in, el * 2 * 128 * 128 + dp * 128 * 128,
                              [[128, 128], [EL * 2 * 128 * 128, NCORES], [1, 128]]),
                        in_=eo.rearrange("p (j b) -> p j b", j=NCORES))

        # =============== AllToAll ===============
        nc.gpsimd.collective_compute(
            "AllToAll", AluOp.bypass, replica_groups=groups,
            ins=[d_a2a_in.ap().opt()], outs=[d_a2a_out.ap().opt()])

        # =============== Scope 2: gate MLP + top-2 ===============
        with tc.tile_pool(name="gate", bufs=1) as gp, \
             tc.tile_pool(name="gtmp", bufs=3) as gtmp, \
             tc.tile_pool(name="gchunk", bufs=4) as gch:

            cmt = gp.tile([128, 4, BL], F32)
            nc.sync.dma_start(out=cmt, in_=A(d_cmyT, 0, [[BL, 128], [128 * BL, 4], [1, BL]]))
            g1c = gp.tile([128, 4, 256], F32)
            nc.sync.dma_start(out=g1c, in_=A(d_gW1c, 0, [[256, 128], [128 * 256, 4], [1, 256]]))
            gmT = gp.tile([128, 2], F32)
            nc.sync.dma_start(out=gmT, in_=A(d_gW1m, 0, [[1, 128], [128, 2]]))
            pptT = gp.tile([128, 2, S], F32)
            nc.sync.dma_start(out=pptT, in_=A(d_pptT, 0, [[S, 128], [128 * S, 2], [1, S]]))
            gw2 = gp.tile([128, 2, E], F32)
            nc.sync.dma_start(out=gw2, in_=A(d_gW2, 0, [[E, 128], [128 * E, 2], [1, E]]))
            gb2T = gp.tile([16, 1], F32)
            nc.sync.dma_start(out=gb2T, in_=A(d_gb2, 0, [[1, 16], [1, 1]]))
            mask_tok = gp.tile([128, S], F32)   # [p, ch]: maskf_flat[ch*128+p]
            nc.sync.dma_start(out=mask_tok, in_=A(d_maskf, 0, [[1, 128], [128, S]]))

            with tc.tile_pool(name="cpps", bufs=1, space="PSUM") as cpps:
                cp_ps = cpps.tile([128, 256], F32, tag="cp")
                for dp in range(2):
                    for kc in range(4):
                        nc.tensor.matmul(cp_ps[:, dp * 128:(dp + 1) * 128],
                                         g1c[:, kc, dp * 128:(dp + 1) * 128],
                                         cmt[:, kc, :],
                                         start=(kc == 0), stop=(kc == 3))
                cp = gp.tile([128, 2, BL], F32)
                nc.scalar.copy(out=cp, in_=cp_ps.rearrange("p (a b) -> p a b", a=2))

            ltT = gp.tile([16, T], F32)
            with tc.tile_pool(name="lpps", bufs=2, space="PSUM") as lpps:
                # chunks of 512 tokens (= 8 b rows each)
                for ct in range(16):
                    mrep = gch.tile([128, 512], F32, tag="mrep")
                    nc.gpsimd.dma_start(out=mrep,
                                        in_=A(d_maskf, ct * 512, [[0, 128], [1, 512]]))
                    pres = []
                    for dp in range(2):
                        pre = gch.tile([128, 512], F32, tag=f"pre{dp}")
                        prev = pre.rearrange("p (b s) -> p b s", b=8)
                        # (mask * gW1m[d]) + cpart[d, b]
                        nc.vector.scalar_tensor_tensor(
                            prev, mrep.rearrange("p (b s) -> p b s", b=8),
                            gmT[:, dp:dp + 1],
                            cp[:, dp, ct * 8:(ct + 1) * 8].unsqueeze(2).broadcast_to([128, 8, 64]),
                            op0=AluOp.mult, op1=AluOp.add)
                        # + pptT[d, s]
                        nc.vector.tensor_tensor(
                            prev, prev,
                            pptT[:, dp, :].unsqueeze(1).broadcast_to([128, 8, 64]),
                            op=AluOp.add)
                        nc.scalar.activation(pre, pre, Act.Gelu)
                        pres.append(pre)
                    lp = lpps.tile([16, 512], F32, tag="lp")
                    for dp in range(2):
                        nc.tensor.matmul(lp, gw2[:, dp, :], pres[dp],
                                         start=(dp == 0), stop=(dp == 1))
                    nc.scalar.activation(ltT[:, ct * 512:(ct + 1) * 512], lp,
                                         Act.Identity, bias=gb2T[:, 0:1])

            # transpose to token-major LT[p, ch, e], p+128*ch = token
            ident = gp.tile([16, 16], F32)
            nc.gpsimd.memset(ident, 0.0)
            nc.gpsimd.affine_select(out=ident, in_=ident,
                                    compare_op=AluOp.not_equal, fill=1.0,
                                    base=0, pattern=[[-1, 16]],
                                    channel_multiplier=1)
            LT = gp.tile([128, S, E], F32)
            with tc.tile_pool(name="tps", bufs=4, space="PSUM") as tps:
                for ch in range(S):
                    tp = tps.tile([128, 16], F32, tag="tp")
                    nc.tensor.transpose(tp, ltT[:, ch * 128:(ch + 1) * 128], ident)
                    nc.vector.tensor_copy(LT[:, ch, :], tp)

            # sort key replicating jax top_k's -0.0 < +0.0 total order:
            # key = raw*mask + (raw > 0) * 1e-30
            pos = gp.tile([128, S, E], F32)
            nc.vector.tensor_scalar(pos, LT, 0.0, 1e-30,
                                    op0=AluOp.is_gt, op1=AluOp.mult)
            nc.vector.tensor_tensor(LT, LT,
                                    mask_tok.unsqueeze(2).broadcast_to([128, S, E]),
                                    op=AluOp.mult)
            nc.vector.tensor_add(LT, LT, pos)

            # ---- top-2 ----
            revio_i = gp.tile([128, S * E], I32)
            nc.gpsimd.iota(revio_i, [[0, S], [-1, E]], base=E, channel_multiplier=0)
            revio = gp.tile([128, S, E], F32)
            nc.vector.tensor_copy(revio, revio_i.rearrange("p (s e) -> p s e", s=S))
            eio_i = gp.tile([128, S * E], I32)
            nc.gpsimd.iota(eio_i, [[0, S], [1, E]], base=0, channel_multiplier=0)
            eio = gp.tile([128, S, E], F32)
            nc.vector.tensor_copy(eio, eio_i.rearrange("p (s e) -> p s e", s=S))

            def bce(x):  # [128, S] -> [128, S, E] broadcast
                return x.unsqueeze(2).broadcast_to([128, S, E])

            m1 = gp.tile([128, S], F32)
            nc.vector.tensor_reduce(m1, LT, mybir.AxisListType.X, AluOp.max)
            t1 = gtmp.tile([128, S, E], F32, tag="t3d")
            nc.vector.tensor_tensor(t1, LT, bce(m1), op=AluOp.is_equal)
            nc.vector.tensor_mul(t1, t1, revio)
            i1f = gp.tile([128, S], F32)
            nc.vector.tensor_reduce(i1f, t1, mybir.AxisListType.X, AluOp.max)
            nc.vector.tensor_scalar(i1f, i1f, -1.0, float(E),
                                    op0=AluOp.mult, op1=AluOp.add)
            t2 = gtmp.tile([128, S, E], F32, tag="t3d")
            nc.vector.tensor_tensor(t2, eio, bce(i1f), op=AluOp.is_equal)
            l2 = gtmp.tile([128, S, E], F32, tag="t3d")
            nc.vector.scalar_tensor_tensor(l2, t2, -1e30, LT,
                                           op0=AluOp.mult, op1=AluOp.add)
            m2 = gp.tile([128, S], F32)
            nc.vector.tensor_reduce(m2, l2, mybir.AxisListType.X, AluOp.max)
            t3 = gtmp.tile([128, S, E], F32, tag="t3d")
            nc.vector.tensor_tensor(t3, l2, bce(m2), op=AluOp.is_equal)
            nc.vector.tensor_mul(t3, t3, revio)
            i2f = gp.tile([128, S], F32)
            nc.vector.tensor_reduce(i2f, t3, mybir.AxisListType.X, AluOp.max)
            nc.vector.tensor_scalar(i2f, i2f, -1.0, float(E),
                                    op0=AluOp.mult, op1=AluOp.add)
            dm = gp.tile([128, S], F32)
            nc.vector.tensor_sub(dm, m1, m2)
            g1 = gp.tile([128, S], F32)
            nc.scalar.activation(g1, dm, Act.Sigmoid)
            g2 = gp.tile([128, S], F32)
            nc.vector.tensor_scalar(g2, g1, -1.0, 1.0,
                                    op0=AluOp.mult, op1=AluOp.add)
            i1i = gp.tile([128, S], I32)
            nc.vector.tensor_copy(i1i, i1f)
            i2i = gp.tile([128, S], I32)
            nc.vector.tensor_copy(i2i, i2f)

            # token-major [p, ch]; host converts to t-order via transpose
            nc.sync.dma_start(out=d_g1o.ap(), in_=g1)
            nc.sync.dma_start(out=d_g2o.ap(), in_=g2)
            nc.sync.dma_start(out=d_i1o.ap(), in_=i1i)
            nc.sync.dma_start(out=d_i2o.ap(), in_=i2i)
            nc.sync.dma_start(out=d_g1s.ap(), in_=g1)
            nc.sync.dma_start(out=d_g2s.ap(), in_=g2)
            nc.sync.dma_start(out=d_i1s.ap(), in_=i1f)
            nc.sync.dma_start(out=d_i2s.ap(), in_=i2f)

        # =============== Scope 3: wT, combine, decode ===============
        with tc.tile_pool(name="cmb", bufs=1) as cmb:

            # wT[e, p, ch] = g1*(e==i1) + g2*(e==i2), u-order (p-major)
            iotaP_i = cmb.tile([16, 1], I32)
            nc.gpsimd.iota(iotaP_i, [[0, 1]], base=0, channel_multiplier=1)
            iotaP = cmb.tile([16, 1], F32)
            nc.vector.tensor_copy(iotaP, iotaP_i)
            wT = cmb.tile([16, 128, S], F32)
            combT = cmb.tile([128, BL, 2, S], F32)  # [d, b, dp, s]

            WC = 1024
            with tc.tile_pool(name="wtmp", bufs=2) as wtmp:
                for wc in range(T // WC):
                    i1r = wtmp.tile([16, WC], F32, tag="i1r")
                    nc.gpsimd.dma_start(out=i1r, in_=A(d_i1s, wc * WC, [[0, 16], [1, WC]]))
                    i2r = wtmp.tile([16, WC], F32, tag="i2r")
                    nc.gpsimd.dma_start(out=i2r, in_=A(d_i2s, wc * WC, [[0, 16], [1, WC]]))
                    g1r = wtmp.tile([16, WC], F32, tag="g1r")
                    nc.gpsimd.dma_start(out=g1r, in_=A(d_g1s, wc * WC, [[0, 16], [1, WC]]))
                    g2r = wtmp.tile([16, WC], F32, tag="g2r")
                    nc.gpsimd.dma_start(out=g2r, in_=A(d_g2s, wc * WC, [[0, 16], [1, WC]]))
                    wv = wT.rearrange("e p c -> e (p c)")[:, wc * WC:(wc + 1) * WC]
                    tmp = wtmp.tile([16, WC], F32, tag="wtm")
                    nc.vector.scalar_tensor_tensor(wv, i1r, iotaP[:, 0:1], g1r,
                                                   op0=AluOp.is_equal, op1=AluOp.mult)
                    nc.vector.scalar_tensor_tensor(tmp, i2r, iotaP[:, 0:1], g2r,
                                                   op0=AluOp.is_equal, op1=AluOp.mult)
                    nc.vector.tensor_add(wv, wv, tmp)

            BBLK = 16
            with tc.tile_pool(name="eblk", bufs=2) as eblk, \
                 tc.tile_pool(name="cps", bufs=4, space="PSUM") as cps:
                for blk in range(BL // BBLK):
                    Esb = eblk.tile([16, 2, 128, BBLK], F32, tag="Esb")
                    nc.sync.dma_start(
                        out=Esb,
                        in_=A(d_a2a_out, blk * BBLK,
                              [[2 * 128 * BL, 16], [128 * BL, 2], [BL, 128], [1, BBLK]]))
                    for bb in range(BBLK):
                        b = blk * BBLK + bb
                        cb = cps.tile([128, 2, S], F32, tag="cb")
                        rhs = wT[:, (b % 2) * 64:(b % 2) * 64 + 64, b // 2]
                        for dp in range(2):
                            nc.tensor.matmul(cb[:, dp, :], Esb[:, dp, :, bb], rhs,
                                             start=True, stop=True)
                        nc.scalar.copy(out=combT[:, b, :, :], in_=cb)

            # ---- decode ----
            with tc.tile_pool(name="dec", bufs=1) as dec, \
                 tc.tile_pool(name="dcw", bufs=2) as dcw, \
                 tc.tile_pool(name="dstg", bufs=3) as dstg, \
                 tc.tile_pool(name="dps", bufs=2, space="PSUM") as dps:
                nwt = dec.tile([128, 2, NUM], F32)
                nc.sync.dma_start(out=nwt, in_=A(d_nWT, 0, [[NUM, 128], [128 * NUM, 2], [1, NUM]]))
                nbt = dec.tile([1, NUM], F32)
                nc.sync.dma_start(out=nbt, in_=d_nb.ap())
                cbt = dec.tile([V, NUM], F32)
                nc.sync.dma_start(out=cbt, in_=d_cbT.ap())

                num_sb = dec.tile([1, NUM * BL], F32)
                for n in range(NUM):
                    pn = dps.tile([1, BL], F32, tag="pn")
                    for dp in range(2):
                        nc.tensor.matmul(pn, nwt[:, dp, n:n + 1],
                                         combT[:, :, dp, n],
                                         start=(dp == 0), stop=(dp == 1))
                    nc.scalar.activation(num_sb[:, n * BL:(n + 1) * BL], pn,
                                         Act.Identity, bias=nbt[0:1, n:n + 1])
                nc.sync.dma_start(out=d_numT.ap(),
                                  in_=num_sb.rearrange("p (n b) -> p n b", n=NUM))

                NBLK = 8
                for ng in range(NUM // NBLK):
                    cwt = dcw.tile([128, 2, NBLK, V], F32, tag="cwt")
                    for dp in range(2):
                        nc.sync.dma_start(
                            out=cwt[:, dp, :, :],
                            in_=A(d_cW, ng * NBLK * D * V + dp * 128 * V,
                                  [[V, 128], [D * V, NBLK], [1, V]]))
                    for nn in range(NBLK):
                        n = ng * NBLK + nn
                        pc = dps.tile([V, BL], F32, tag="pc")
                        for dp in range(2):
                            nc.tensor.matmul(pc, cwt[:, dp, nn, :],
                                             combT[:, :, dp, NUM + n],
                                             start=(dp == 0), stop=(dp == 1))
                        cstg = dstg.tile([V, BL], F32, tag="cstg")
                        nc.scalar.activation(cstg, pc, Act.Identity,
                                             bias=cbt[:, n:n + 1])
                        nc.sync.dma_start(
                            out=A(d_catT, n * BL, [[NUM * BL, V], [1, BL]]),
                            in_=cstg)

    nc.compile()
    return nc


_NC_CACHE = []


def kernel(codec, mask_pos, pos_emb, type_emb, gW1, gb1, gW2, gb2,
           eW0, eb0, eW1, eb1, eW2, eb2, nW, nb, cW, cb):
    codec = np.ascontiguousarray(np.asarray(codec, np.float32))
    mask_pos = np.asarray(mask_pos, np.int32)
    gW1 = np.asarray(gW1, np.float32)
    is_cat = (np.arange(S) >= NUM).astype(np.int32)
    ppt = (np.asarray(pos_emb, np.float32) @ gW1[C:2 * C]
           + np.asarray(type_emb, np.float32)[is_cat] @ gW1[2 * C:3 * C]
           + np.asarray(gb1, np.float32))                       # [S, 256]

    codecT = np.ascontiguousarray(codec.T)                      # [C, B]
    common = {
        "codecT": codecT,
        "pptT": np.ascontiguousarray(ppt.T),
        "gW1c": np.ascontiguousarray(gW1[:C]),
        "gW1m": np.ascontiguousarray(gW1[3 * C:3 * C + 1]),
        "gW2": np.ascontiguousarray(np.asarray(gW2, np.float32)),
        "gb2": np.ascontiguousarray(np.asarray(gb2, np.float32)[None, :]),
        "nWT": np.ascontiguousarray(np.asarray(nW, np.float32).T),
        "nb": np.ascontiguousarray(np.asarray(nb, np.float32)[None, :]),
        "cW": np.ascontiguousarray(np.asarray(cW, np.float32)),
        "cbT": np.ascontiguousarray(np.asarray(cb, np.float32).T),
    }
    eW0 = np.asarray(eW0, np.float32); eb0 = np.asarray(eb0, np.float32)
    eW1 = np.asarray(eW1, np.float32); eb1 = np.asarray(eb1, np.float32)
    eW2 = np.asarray(eW2, np.float32); eb2 = np.asarray(eb2, np.float32)
    maskf = mask_pos.astype(np.float32)

    in_maps = []
    for i in range(NCORES):
        m = dict(common)
        m["codec_myT"] = np.ascontiguousarray(codecT[:, i * BL:(i + 1) * BL])
        m["maskf"] = np.ascontiguousarray(maskf[i * BL:(i + 1) * BL])
        m["ew0"] = np.ascontiguousarray(eW0[EL * i:EL * i + EL])
        m["eb0"] = np.ascontiguousarray(eb0[EL * i:EL * i + EL])
        m["ew1"] = np.ascontiguousarray(eW1[EL * i:EL * i + EL])
        m["eb1"] = np.ascontiguousarray(eb1[EL * i:EL * i + EL])
        m["ew2"] = np.ascontiguousarray(eW2[EL * i:EL * i + EL])
        m["eb2"] = np.ascontiguousarray(eb2[EL * i:EL * i + EL])
        in_maps.append(m)

    if not _NC_CACHE:
        _NC_CACHE.append(_build_nc())
    nc = _NC_CACHE[0]

    res = run_bass_kernel_spmd(nc, in_maps, core_ids=list(range(NCORES)))
    kernel._last_result = res

    num_l, cat_l, g_l, t_l = [], [], [], []
    for i in range(NCORES):
        r = res.results[i]
        num_l.append(np.ascontiguousarray(r["numT_out"].T))          # [BL, 32]
        cat_l.append(np.ascontiguousarray(r["catT_out"].transpose(2, 1, 0)))

        def torder(a):
            return np.ascontiguousarray(a.T).reshape(BL, S)
        g_l.append(np.stack([torder(r["g1_out"]), torder(r["g2_out"])], -1))
        t_l.append(np.stack([torder(r["i1_out"]), torder(r["i2_out"])], -1)
                   .astype(np.int32))

    num_recon = np.concatenate(num_l, 0)
    cat_recon = np.concatenate(cat_l, 0)
    gates = np.concatenate(g_l, 0).astype(np.float32)
    topi = np.concatenate(t_l, 0)
    return num_recon, cat_recon, gates, topi, mask_pos
